# revision 1
# baseline (speedup 1.0000x reference)
"""DictionaryLearningOMP forward on 8 TRN2 NeuronCores.

Reference computes out = (pinv(D) @ X).T with D = dictionary.T [256,512],
X = z_e [256,65536].  Equivalently out = X.T @ pinv(dictionary), where
pinv(dictionary) is [256,512].

Sharding: data-parallel along the N=65536 column dim -> 8 shards of 8192
columns.  The small [256,512] pinverse is computed once on host and
replicated to every core.  Each core does a [8192,256]x[256,512] matmul
(out tile = x_tile.T @ dpt via the PE array) and writes its [8192,512]
slice; host concatenates.
"""

import os

import numpy as np

import concourse.bacc as bacc
import concourse.bass as bass
import concourse.mybir as mybir
import concourse.tile as tile
from concourse.bass_utils import run_bass_kernel_spmd

DIM = 256  # contraction dim (data dimension)
KATOMS = 512  # codebook size (output cols)
NTOT = 65536  # total signal columns
NCORES = 8
NSHARD = NTOT // NCORES  # 8192 columns per core

NB = 512  # x-load tile width (columns of N per DMA)
SUB = 128  # psum tile rows (partition dim of an output tile)

# matmul dtype: float32 is exact-ish but 4 cycles/row on the PE;
# float32r streams at 1 cycle/row for free dim >= 256.
_MM_DTYPE_NAME = os.environ.get("KERNEL_MM_DTYPE", "float32")

LAST_RESULT = None  # BassKernelResults of the most recent run (for test.py)

_cache = {}


def _build_module():
    mm_dt = getattr(mybir.dt, _MM_DTYPE_NAME)
    f32 = mybir.dt.float32

    nc = bacc.Bacc("TRN2", target_bir_lowering=False, debug=False)

    x = nc.dram_tensor("x", [DIM, NSHARD], mm_dt, kind="ExternalInput")
    dpt = nc.dram_tensor("dpt", [DIM, KATOMS], mm_dt, kind="ExternalInput")
    out = nc.dram_tensor("out", [NSHARD, KATOMS], f32, kind="ExternalOutput")

    with tile.TileContext(nc) as tc:
        with (
            tc.tile_pool(name="dict", bufs=1) as dict_pool,
            tc.tile_pool(name="xin", bufs=3) as xin_pool,
            tc.tile_pool(name="outs", bufs=4) as out_pool,
            tc.tile_pool(name="psum", bufs=4, space=bass.MemorySpace.PSUM) as psum_pool,
        ):
            dpt_sb = dict_pool.tile([128, 2, KATOMS], mm_dt)
            for j in range(2):
                nc.sync.dma_start(dpt_sb[:, j, :], dpt[j * 128 : (j + 1) * 128, :])

            for n0 in range(0, NSHARD, NB):
                xt = xin_pool.tile([128, 2, NB], mm_dt)
                for j in range(2):
                    nc.sync.dma_start(
                        xt[:, j, :], x[j * 128 : (j + 1) * 128, n0 : n0 + NB]
                    )
                for s in range(NB // SUB):
                    ps = psum_pool.tile([SUB, KATOMS], f32)
                    nc.tensor.matmul(
                        ps[:],
                        xt[:, 0, s * SUB : (s + 1) * SUB],
                        dpt_sb[:, 0, :],
                        start=True,
                        stop=False,
                    )
                    nc.tensor.matmul(
                        ps[:],
                        xt[:, 1, s * SUB : (s + 1) * SUB],
                        dpt_sb[:, 1, :],
                        start=False,
                        stop=True,
                    )
                    ot = out_pool.tile([SUB, KATOMS], f32)
                    nc.vector.tensor_copy(ot[:], ps[:])
                    n1 = n0 + s * SUB
                    nc.sync.dma_start(out[n1 : n1 + SUB, :], ot[:])

    nc.compile()
    return nc


def _get_module():
    if "nc" not in _cache:
        _cache["nc"] = _build_module()
    return _cache["nc"]


def kernel(z_e, dictionary):
    z_e = np.asarray(z_e, dtype=np.float32)
    dictionary = np.asarray(dictionary, dtype=np.float32)
    assert z_e.shape == (DIM, NTOT), z_e.shape
    assert dictionary.shape == (KATOMS, DIM), dictionary.shape

    # pinv(D).T = pinv(D.T) = pinv(dictionary): [256, 512].  Tiny; done in
    # f64 on host once, replicated to all cores.
    dpt = np.linalg.pinv(dictionary.astype(np.float64)).astype(np.float32)
    dpt = np.ascontiguousarray(dpt)

    nc = _get_module()
    in_maps = [
        {
            "x": np.ascontiguousarray(z_e[:, i * NSHARD : (i + 1) * NSHARD]),
            "dpt": dpt,
        }
        for i in range(NCORES)
    ]
    res = run_bass_kernel_spmd(nc, in_maps, core_ids=list(range(NCORES)))
    global LAST_RESULT
    LAST_RESULT = res
    return np.concatenate([r["out"] for r in res.results], axis=0)


# revision 3
# speedup vs baseline: 1.6694x; 1.6694x over previous
"""DictionaryLearningOMP forward on 8 TRN2 NeuronCores.

Reference computes out = (pinv(D) @ X).T with D = dictionary.T [256,512],
X = z_e [256,65536].  Equivalently out = X.T @ pinv(dictionary), where
pinv(dictionary) is [256,512].

Sharding: data-parallel along the N=65536 column dim -> 8 shards of 8192
columns.  The small [256,512] pinverse is computed once on host (f64) and
replicated to every core.  Each core computes out_shard[8192,512] =
x_shard.T @ dpt on the PE array (contract dim 256 = 2x128 chunks,
PSUM tiles [128,512]) and writes its slice; host concatenates.

Precision modes (KERNEL_MODE env; shipped default below):
  f16     in f16 / f16 matmul / out f16 (host upcasts)   ~12 MB DMA per core
  f32r    in f32 / float32r matmul / out f32             ~24.5 MB per core
  f32     in f32 / float32 matmul / out f32 (4x PE cost)
  bf16x3  in bf16 hi+lo / 3-way split matmul / out f32   (~fp32 accuracy)
"""

import os

import numpy as np

import concourse.bacc as bacc
import concourse.bass as bass
import concourse.mybir as mybir
import concourse.tile as tile
from concourse.bass_utils import run_bass_kernel_spmd

DIM = 256  # contraction dim (data dimension)
KATOMS = 512  # codebook size (output cols)
NTOT = 65536  # total signal columns
NCORES = 8
NSHARD = NTOT // NCORES  # 8192 columns per core

MODE = os.environ.get("KERNEL_MODE", "f16")

LAST_RESULT = None  # BassKernelResults of the most recent run (for test.py)

_cache = {}


def _mode_cfg(mode):
    dt = mybir.dt
    if mode == "f16":
        # in f16, out f16; 1MB loads ([128,2,2048] f16), 1MB stores (G=8)
        return dict(in_dt=dt.float16, out_dt=dt.float16, nterms=1, nbig=2048, g=8)
    if mode == "f32r":
        return dict(in_dt=dt.float32r, out_dt=dt.float32, nterms=1, nbig=1024, g=4)
    if mode == "f32":
        return dict(in_dt=dt.float32, out_dt=dt.float32, nterms=1, nbig=1024, g=4)
    if mode == "bf16x3":
        return dict(in_dt=dt.bfloat16, out_dt=dt.float32, nterms=3, nbig=2048, g=4)
    raise ValueError(mode)


def _build_module(mode):
    cfg = _mode_cfg(mode)
    in_dt, out_dt = cfg["in_dt"], cfg["out_dt"]
    NBIG, G = cfg["nbig"], cfg["g"]
    f32 = mybir.dt.float32
    nterms = cfg["nterms"]
    # term list: for split modes, (x_idx, d_idx) operand pairs to accumulate
    terms = [(0, 0)] if nterms == 1 else [(0, 0), (1, 0), (0, 1)]
    nxa = 2 if nterms > 1 else 1  # number of x input arrays (hi/lo)
    nda = 2 if nterms > 1 else 1

    nc = bacc.Bacc("TRN2", target_bir_lowering=False, debug=False)

    xs = [
        nc.dram_tensor(f"x{i}", [DIM, NSHARD], in_dt, kind="ExternalInput")
        for i in range(nxa)
    ]
    dps = [
        nc.dram_tensor(f"dpt{i}", [DIM, KATOMS], in_dt, kind="ExternalInput")
        for i in range(nda)
    ]
    out = nc.dram_tensor("out", [NSHARD, KATOMS], out_dt, kind="ExternalOutput")

    # fold the two 128-row contraction chunks into the partition dim
    xs_v = [x.rearrange("(j p) n -> p j n", p=128) for x in xs]
    out_v = out.rearrange("(m g p) k -> m p g k", p=128, g=G)

    n_sub = NBIG // 128  # psum tiles per x load
    with tile.TileContext(nc) as tc:
        with (
            tc.tile_pool(name="dict", bufs=1) as dict_pool,
            tc.tile_pool(name="xin", bufs=3) as xin_pool,
            tc.tile_pool(name="outs", bufs=3) as out_pool,
            tc.tile_pool(name="psum", bufs=8, space=bass.MemorySpace.PSUM) as psum_pool,
        ):
            dpt_sbs = []
            for i, dp in enumerate(dps):
                dpt_sb = dict_pool.tile([128, 2, KATOMS], in_dt, tag=f"dict{i}")
                nc.sync.dma_start(dpt_sb[:], dp.rearrange("(j p) k -> p j k", p=128))
                dpt_sbs.append(dpt_sb)

            gi = 0  # index within current output group
            ot = None
            for n0 in range(0, NSHARD, NBIG):
                xts = []
                for i, xv in enumerate(xs_v):
                    xt = xin_pool.tile([128, 2, NBIG], in_dt, tag=f"x{i}")
                    nc.sync.dma_start(xt[:], xv[:, :, n0 : n0 + NBIG])
                    xts.append(xt)
                for s in range(n_sub):
                    ps = psum_pool.tile([128, KATOMS], f32)
                    nmm = len(terms) * 2
                    mi = 0
                    for xi, di in terms:
                        for j in range(2):
                            nc.tensor.matmul(
                                ps[:],
                                xts[xi][:, j, s * 128 : (s + 1) * 128],
                                dpt_sbs[di][:, j, :],
                                start=(mi == 0),
                                stop=(mi == nmm - 1),
                            )
                            mi += 1
                    if gi == 0:
                        ot = out_pool.tile([128, G, KATOMS], out_dt, tag="ot")
                    # split psum->sbuf copies between DVE and ACT
                    if (gi % 4) != 3:
                        nc.vector.tensor_copy(ot[:, gi, :], ps[:])
                    else:
                        nc.scalar.copy(ot[:, gi, :], ps[:])
                    gi += 1
                    if gi == G:
                        m = (n0 + (s + 1) * 128) // (G * 128) - 1
                        nc.scalar.dma_start(out_v[m], ot[:])
                        gi = 0

    nc.compile()
    return nc


def _get_module(mode):
    if mode not in _cache:
        _cache[mode] = _build_module(mode)
    return _cache[mode]


def _split_hi_lo(a, dtype):
    hi = a.astype(dtype)
    lo = (a - hi.astype(np.float32)).astype(dtype)
    return hi, lo


def kernel(z_e, dictionary):
    import ml_dtypes

    z_e = np.asarray(z_e, dtype=np.float32)
    dictionary = np.asarray(dictionary, dtype=np.float32)
    assert z_e.shape == (DIM, NTOT), z_e.shape
    assert dictionary.shape == (KATOMS, DIM), dictionary.shape

    # pinv(D).T = pinv(D.T) = pinv(dictionary): [256, 512].  Tiny; computed
    # in f64 on host once, replicated to all cores.
    dpt = np.linalg.pinv(dictionary.astype(np.float64)).astype(np.float32)

    mode = MODE
    nc = _get_module(mode)

    if mode == "f16":
        xs = [z_e.astype(np.float16)]
        dps = [dpt.astype(np.float16)]
    elif mode == "bf16x3":
        xh, xl = _split_hi_lo(z_e, ml_dtypes.bfloat16)
        dh, dl = _split_hi_lo(dpt, ml_dtypes.bfloat16)
        xs = [xh, xl]
        dps = [dh, dl]
    else:
        xs = [z_e]
        dps = [np.ascontiguousarray(dpt)]

    in_maps = []
    for i in range(NCORES):
        m = {}
        for k, x in enumerate(xs):
            m[f"x{k}"] = np.ascontiguousarray(x[:, i * NSHARD : (i + 1) * NSHARD])
        for k, dp in enumerate(dps):
            m[f"dpt{k}"] = np.ascontiguousarray(dp)
        in_maps.append(m)

    res = run_bass_kernel_spmd(nc, in_maps, core_ids=list(range(NCORES)))
    global LAST_RESULT
    LAST_RESULT = res
    outs = [r["out"] for r in res.results]
    full = np.concatenate(outs, axis=0)
    if full.dtype != np.float32:
        full = full.astype(np.float32)
    return full


# revision 6
# speedup vs baseline: 1.7168x; 1.0284x over previous
"""DictionaryLearningOMP forward on 8 TRN2 NeuronCores.

Reference computes out = (pinv(D) @ X).T with D = dictionary.T [256,512],
X = z_e [256,65536].  Equivalently out = X.T @ pinv(dictionary), where
pinv(dictionary) is [256,512].

Sharding: data-parallel along the N=65536 column dim -> 8 shards of 8192
columns.  The small [256,512] pinverse is computed once on host (f64) and
replicated to every core.  Each core computes out_shard[8192,512] =
x_shard.T @ dpt on the PE array (contract dim 256 = 2x128 chunks,
PSUM tiles [128,512]) and writes its slice; host concatenates.

Precision modes (KERNEL_MODE env; shipped default below):
  f16     in f16 / f16 matmul / out f16 (host upcasts)   ~12 MB DMA per core
  f32r    in f32 / float32r matmul / out f32             ~24.5 MB per core
  f32     in f32 / float32 matmul / out f32 (4x PE cost)
  bf16x3  in bf16 hi+lo / 3-way split matmul / out f32   (~fp32 accuracy)
"""

import os

import numpy as np

import concourse.bacc as bacc
import concourse.bass as bass
import concourse.mybir as mybir
import concourse.tile as tile
from concourse.bass_utils import run_bass_kernel_spmd

DIM = 256  # contraction dim (data dimension)
KATOMS = 512  # codebook size (output cols)
NTOT = 65536  # total signal columns
NCORES = 8
NSHARD = NTOT // NCORES  # 8192 columns per core

MODE = os.environ.get("KERNEL_MODE", "f16")

LAST_RESULT = None  # BassKernelResults of the most recent run (for test.py)

_cache = {}


def _mode_cfg(mode):
    dt = mybir.dt
    if mode == "f16":
        # in f16, out f16; 1MB loads ([128,2,2048] f16), 512KB stores (G=4)
        return dict(in_dt=dt.float16, out_dt=dt.float16, nterms=1, nbig=2048, g=4)
    if mode == "f32r":
        return dict(in_dt=dt.float32r, out_dt=dt.float32, nterms=1, nbig=1024, g=4)
    if mode == "f32":
        return dict(in_dt=dt.float32, out_dt=dt.float32, nterms=1, nbig=1024, g=4)
    if mode == "bf16x3":
        return dict(in_dt=dt.bfloat16, out_dt=dt.float32, nterms=3, nbig=2048, g=4)
    raise ValueError(mode)


def _build_module(mode):
    cfg = _mode_cfg(mode)
    in_dt, out_dt = cfg["in_dt"], cfg["out_dt"]
    NBIG, G = cfg["nbig"], cfg["g"]
    f32 = mybir.dt.float32
    nterms = cfg["nterms"]
    # term list: for split modes, (x_idx, d_idx) operand pairs to accumulate
    terms = [(0, 0)] if nterms == 1 else [(0, 0), (1, 0), (0, 1)]
    nxa = 2 if nterms > 1 else 1  # number of x input arrays (hi/lo)
    nda = 2 if nterms > 1 else 1

    nc = bacc.Bacc("TRN2", target_bir_lowering=False, debug=False)

    xs = [
        nc.dram_tensor(f"x{i}", [DIM, NSHARD], in_dt, kind="ExternalInput")
        for i in range(nxa)
    ]
    dps = [
        nc.dram_tensor(f"dpt{i}", [DIM, KATOMS], in_dt, kind="ExternalInput")
        for i in range(nda)
    ]
    out = nc.dram_tensor("out", [NSHARD, KATOMS], out_dt, kind="ExternalOutput")

    # fold the two 128-row contraction chunks into the partition dim
    xs_v = [x.rearrange("(j p) n -> p j n", p=128) for x in xs]
    out_v = out.rearrange("(m g p) k -> m p g k", p=128, g=G)

    n_sub = NBIG // 128  # psum tiles per x load
    with tile.TileContext(nc) as tc:
        with (
            tc.tile_pool(name="dict", bufs=1) as dict_pool,
            tc.tile_pool(name="xin", bufs=4) as xin_pool,
            tc.tile_pool(name="outs", bufs=4) as out_pool,
            tc.tile_pool(name="psum", bufs=8, space=bass.MemorySpace.PSUM) as psum_pool,
        ):
            dpt_sbs = []
            for i, dp in enumerate(dps):
                dpt_sb = dict_pool.tile([128, 2, KATOMS], in_dt, tag=f"dict{i}")
                nc.sync.dma_start(dpt_sb[:], dp.rearrange("(j p) k -> p j k", p=128))
                dpt_sbs.append(dpt_sb)

            # split the first load chunk small so the PE/copy/store pipeline
            # primes as early as possible
            chunks = []
            pos = 0
            for w in [512, NBIG - 512] + [NBIG] * (NSHARD // NBIG - 1):
                chunks.append((pos, w))
                pos += w

            gi = 0  # index within current output group
            ot = None
            tiles_done = 0
            for ci, (n0, w) in enumerate(chunks):
                xts = []
                for i, xv in enumerate(xs_v):
                    xt = xin_pool.tile([128, 2, w], in_dt, tag=f"x{i}")
                    nc.sync.dma_start(xt[:], xv[:, :, n0 : n0 + w])
                    xts.append(xt)
                for s in range(w // 128):
                    ps = psum_pool.tile([128, KATOMS], f32)
                    nmm = len(terms) * 2
                    mi = 0
                    for xi, di in terms:
                        for j in range(2):
                            nc.tensor.matmul(
                                ps[:],
                                xts[xi][:, j, s * 128 : (s + 1) * 128],
                                dpt_sbs[di][:, j, :],
                                start=(mi == 0),
                                stop=(mi == nmm - 1),
                            )
                            mi += 1
                    if gi == 0:
                        ot = out_pool.tile([128, G, KATOMS], out_dt, tag="ot")
                    # split psum->sbuf copies evenly between DVE and ACT
                    if (gi % 2) == 0:
                        nc.vector.tensor_copy(ot[:, gi, :], ps[:])
                    else:
                        nc.scalar.copy(ot[:, gi, :], ps[:])
                    gi += 1
                    tiles_done += 1
                    if gi == G:
                        m = tiles_done // G - 1
                        nc.scalar.dma_start(out_v[m], ot[:])
                        gi = 0

    nc.compile()
    return nc


def _get_module(mode):
    if mode not in _cache:
        _cache[mode] = _build_module(mode)
    return _cache[mode]


def _split_hi_lo(a, dtype):
    hi = a.astype(dtype)
    lo = (a - hi.astype(np.float32)).astype(dtype)
    return hi, lo


def kernel(z_e, dictionary):
    import ml_dtypes

    z_e = np.asarray(z_e, dtype=np.float32)
    dictionary = np.asarray(dictionary, dtype=np.float32)
    assert z_e.shape == (DIM, NTOT), z_e.shape
    assert dictionary.shape == (KATOMS, DIM), dictionary.shape

    # pinv(D).T = pinv(D.T) = pinv(dictionary): [256, 512].  Tiny; computed
    # in f64 on host once, replicated to all cores.
    dpt = np.linalg.pinv(dictionary.astype(np.float64)).astype(np.float32)

    mode = MODE
    nc = _get_module(mode)

    if mode == "f16":
        xs = [z_e.astype(np.float16)]
        dps = [dpt.astype(np.float16)]
    elif mode == "bf16x3":
        xh, xl = _split_hi_lo(z_e, ml_dtypes.bfloat16)
        dh, dl = _split_hi_lo(dpt, ml_dtypes.bfloat16)
        xs = [xh, xl]
        dps = [dh, dl]
    else:
        xs = [z_e]
        dps = [np.ascontiguousarray(dpt)]

    in_maps = []
    for i in range(NCORES):
        m = {}
        for k, x in enumerate(xs):
            m[f"x{k}"] = np.ascontiguousarray(x[:, i * NSHARD : (i + 1) * NSHARD])
        for k, dp in enumerate(dps):
            m[f"dpt{k}"] = np.ascontiguousarray(dp)
        in_maps.append(m)

    res = run_bass_kernel_spmd(nc, in_maps, core_ids=list(range(NCORES)))
    global LAST_RESULT
    LAST_RESULT = res
    outs = [r["out"] for r in res.results]
    full = np.concatenate(outs, axis=0)
    if full.dtype != np.float32:
        full = full.astype(np.float32)
    return full
